# revision 11
# baseline (speedup 1.0000x reference)
"""Trainium2 Bass kernel for nn_MultiHeadedAttention (B=2, S=2048, D=1024, H=16).

Sharding: batch (2) x head-groups (4) -> 8 cores. Core c handles batch c//4,
heads [4*(c%4), 4*(c%4)+4).

v2: software-pipelined emission. The Scalar engine's exp stream (128 tiles of
[128kpos, 1024q], ~138us total) is the backbone; PE interleaves scores
(producer, 2x512-row matmuls per tile), PV (consumer, lagged LAG tiles), and
all projection / V / output-projection work as filler so PE never idles.
Inputs stream in e-chunk granularity DMAs ordered by first use; output
partials are emitted in bf16 (host sums in fp32).

Per-core math identical to baseline:
  Q^T/K^T projections land heads on partitions 0:64 (rows 64:128 zeroed so
  scores matmuls contract K=128); V in [seq, feat] layout with a ones column
  per head so PV's 65th output row accumulates the softmax denominators.
  exp on ScalarE with the 1/sqrt(dk) scale folded in (no max-subtraction:
  |scores| <~ 8 is safe). Normalization: reciprocal on a [128,8] lane-parallel
  reshape via DRAM bounce + broadcast DMA + one vector multiply.
  t-bias MLP folded into the K bias on host.
"""

import numpy as np

B, S, D, H, DK = 2, 2048, 1024, 16, 64
HPC = 4            # heads per core
DPC = HPC * DK     # 256 features per core
NCORES = 8

TRACE = False          # test harness sets True to capture an NTFF profile
LAST_EXEC_NS = None    # filled when TRACE
LAST_RESULTS = None

_BUILT = None


def _install_ntff_shim():
    """antenv.axon_hooks is absent in this image; recreate it so trace=True
    can ship NTFF profiles back through the axon tunnel."""
    import sys, types
    try:
        from antenv import axon_hooks  # noqa: F401
        return
    except ImportError:
        pass
    import antenv
    mod = types.ModuleType("antenv.axon_hooks")
    _hook = [None]
    mod.set_axon_ntff_profile_hook = lambda h: _hook.__setitem__(0, h)
    mod.get_axon_ntff_profile_hook = lambda: _hook[0]
    sys.modules["antenv.axon_hooks"] = mod
    antenv.axon_hooks = mod
    try:
        from trn_agent_boot.trn_boot import _ntff_profile_via_ctypes
        mod.set_axon_ntff_profile_hook(
            _ntff_profile_via_ctypes("/opt/axon/libaxon_pjrt.so"))
    except Exception:
        pass


def _build():
    """Build the per-core Bass graph (identical on all 8 cores)."""
    import concourse.tile as tile
    from concourse import mybir, bacc

    f32 = mybir.dt.float32
    bf16 = mybir.dt.bfloat16

    nc = bacc.Bacc()

    xq_t = nc.dram_tensor("xq_t", [D, S], bf16, kind="ExternalInput")
    xk_t = nc.dram_tensor("xk_t", [D, S], bf16, kind="ExternalInput")
    xv_t = nc.dram_tensor("xv_t", [D, S], bf16, kind="ExternalInput")
    wq_t = nc.dram_tensor("wq_t", [D, DPC], bf16, kind="ExternalInput")
    wk_t = nc.dram_tensor("wk_t", [D, DPC], bf16, kind="ExternalInput")
    wv_t = nc.dram_tensor("wv_t", [D, DPC], bf16, kind="ExternalInput")
    wo_t = nc.dram_tensor("wo_t", [DPC, D], bf16, kind="ExternalInput")
    bq2 = nc.dram_tensor("bq2", [HPC, DK], f32, kind="ExternalInput")
    bk2 = nc.dram_tensor("bk2", [HPC, DK], f32, kind="ExternalInput")
    bv1 = nc.dram_tensor("bv1", [1, DPC], f32, kind="ExternalInput")
    bo8 = nc.dram_tensor("bo8", [8, 128], f32, kind="ExternalInput")
    y_t = nc.dram_tensor("y_t", [D, S], bf16, kind="ExternalOutput")

    NE = D // 128   # 8 feature chunks
    NST = S // 128  # 16 seq k-tiles of 128
    NPST = 16       # exp staging depth (p tiles in flight)
    LAG = 14        # PV trails the exp backbone by this many tiles

    with tile.TileContext(nc) as tc:
        with tc.tile_pool(name="consts", bufs=1) as consts, \
             tc.tile_pool(name="persist", bufs=1) as persist, \
             tc.tile_pool(name="xq_pool", bufs=1) as xq_pool, \
             tc.tile_pool(name="xk_pool", bufs=2) as xk_pool, \
             tc.tile_pool(name="xv_pool", bufs=2) as xv_pool, \
             tc.tile_pool(name="dnsb", bufs=2) as dn_pool, \
             tc.tile_pool(name="dbsb", bufs=2) as db_pool, \
             tc.tile_pool(name="ysb", bufs=4) as y_pool, \
             tc.tile_pool(name="sc_ps", bufs=2, space="PSUM") as sc_ps, \
             tc.tile_pool(name="o_ps", bufs=1, space="PSUM") as o_ps, \
             tc.tile_pool(name="f_ps", bufs=2, space="PSUM") as f_ps:

            # ---- persistent activations ----
            qt_sb = persist.tile([128, HPC, S], bf16, tag="qt")
            kt_sb = persist.tile([128, HPC, S], bf16, tag="kt")
            # rows 64:128 zeroed so scores matmuls contract K=128
            # (FWL-eligible weights); split across engines so both finish
            # ~7us after queue start and neither blocks chain evacuations.
            nc.gpsimd.memset(qt_sb[64:128, :, :], 0.0)
            nc.vector.memset(kt_sb[64:128, :, :], 0.0)
            v_sb = persist.tile([128, NST, HPC, DK + 1], bf16, tag="v")
            pst = persist.tile([128, NPST, 1024], bf16, tag="pst")
            xa_sb = persist.tile([128, 2, S], bf16, tag="xa")
            ones1 = consts.tile([128, 1], f32, tag="ones1")
            nc.vector.memset(ones1[:, :], 1.0)
            nc.vector.tensor_copy(
                v_sb[:, :, :, DK:DK + 1].rearrange("p a b c -> p (a b c)"),
                ones1[:, 0:1].broadcast_to([128, NST * HPC]))

            # ---- weights + inputs: DMA emission order is priority order ----
            wq_sb = consts.tile([128, NE, DPC], bf16, tag="wq")
            nc.sync.dma_start(wq_sb[:, :, :],
                              wq_t.rearrange("(e p) n -> p e n", p=128))
            x_tiles = {}

            def emit_x_dmas(name, pool, dram, b, halves=1):
                t = pool.tile([128, NE, 1024], bf16, tag=name, name=name)
                x_tiles[(name, b)] = t
                src = dram.rearrange("(e p) s -> p e s", p=128)
                cw = 1024 // halves
                for hf in range(halves):
                    for e in range(NE):
                        nc.sync.dma_start(
                            t[:, e, hf * cw:(hf + 1) * cw],
                            src[:, e, b * 1024 + hf * cw:b * 1024 + (hf + 1) * cw])

            def emit_xv_half(b, hf):
                # column-half loads so V-projection st tiles unblock early
                name = ("xv", b)
                if name not in x_tiles:
                    x_tiles[name] = xv_pool.tile([128, NE, 1024], bf16,
                                                 tag="xv", name="xv")
                t = x_tiles[name]
                src = xv_t.rearrange("(e p) s -> p e s", p=128)
                for e in range(NE):
                    nc.sync.dma_start(
                        t[:, e, hf * 512:(hf + 1) * 512],
                        src[:, e, b * 1024 + hf * 512:b * 1024 + (hf + 1) * 512])

            emit_x_dmas("xq", xq_pool, xq_t, 0)
            wk_sb = consts.tile([128, NE, DPC], bf16, tag="wk")
            nc.sync.dma_start(wk_sb[:, :, :],
                              wk_t.rearrange("(e p) n -> p e n", p=128))
            bq_sb = consts.tile([64, HPC], f32, tag="bq")
            bk_sb = consts.tile([64, HPC], f32, tag="bk")
            nc.sync.dma_start(bq_sb[:, :], bq2.rearrange("h p -> p h"))
            nc.sync.dma_start(bk_sb[:, :], bk2.rearrange("h p -> p h"))
            emit_x_dmas("xk", xk_pool, xk_t, 0)
            emit_x_dmas("xk", xk_pool, xk_t, 1)
            wv_sb = consts.tile([128, NE, DPC], bf16, tag="wv")
            nc.sync.dma_start(wv_sb[:, :, :],
                              wv_t.rearrange("(e p) n -> p e n", p=128))
            bv_bc = consts.tile([128, HPC, DK], f32, tag="bvb")
            nc.sync.dma_start(
                bv_bc.rearrange("p h d -> p (h d)"),
                bv1[0:1, :].broadcast_to([128, DPC]))
            emit_xv_half(0, 0)
            emit_xv_half(0, 1)
            emit_xv_half(1, 0)
            emit_xv_half(1, 1)
            wo_sb = consts.tile([128, 2, D], bf16, tag="wo")
            nc.sync.dma_start(wo_sb[:, :, :],
                              wo_t.rearrange("(f p) n -> p f n", p=128))
            bo_sb = consts.tile([128, 8], f32, tag="bo")
            nc.sync.dma_start(bo_sb[:, :], bo8.rearrange("o p -> p o"))
            # xq block 1 DMAs are deferred (xq_pool bufs=1, SBUF pressure):
            # emitted mid-driver once block-0 Q chains are in the queue.

            # ---- emission state ----
            est = {"pe": 7000.0, "sc": 0.0}
            xq_b1_emitted = [False]
            chain_done = set()   # ('q'|'k', block, m)
            v_done = set()
            pst_slot = {}
            o_tile = [None]

            def emit_chain(kind, b, m):
                if (kind, b, m) in chain_done:
                    return
                chain_done.add((kind, b, m))
                if kind == "q" and b == 1 and ("xq", 1) not in x_tiles:
                    xq_b1_emitted[0] = True
                    emit_x_dmas("xq", xq_pool, xq_t, 1)
                x_t = x_tiles[("xq" if kind == "q" else "xk", b)]
                w_sb = wq_sb if kind == "q" else wk_sb
                dst = qt_sb if kind == "q" else kt_sb
                bias = bq_sb if kind == "q" else bk_sb
                ms = slice(m * 128, m * 128 + 128)
                ps0 = f_ps.tile([128, 512], f32, tag="f", name="fps")
                ps1 = f_ps.tile([128, 512], f32, tag="f", name="fps")
                for e in range(NE):
                    nc.tensor.matmul(ps0[:, :], w_sb[:, e, ms],
                                     x_t[:, e, 0:512],
                                     start=(e == 0), stop=(e == NE - 1))
                    nc.tensor.matmul(ps1[:, :], w_sb[:, e, ms],
                                     x_t[:, e, 512:1024],
                                     start=(e == 0), stop=(e == NE - 1))
                est["pe"] += 16 * 235
                for half, ps in enumerate((ps0, ps1)):
                    sl = slice(b * 1024 + half * 512, b * 1024 + half * 512 + 512)
                    nc.vector.tensor_scalar_add(
                        dst[0:64, 2 * m, sl], ps[0:64, :],
                        bias[:, 2 * m:2 * m + 1])
                    nc.vector.tensor_scalar_add(
                        dst[0:64, 2 * m + 1, sl], ps[64:128, :],
                        bias[:, 2 * m + 1:2 * m + 2])

            def emit_v(st):
                if st in v_done:
                    return
                v_done.add(st)
                b, loc = st // 8, st % 8
                x_t = x_tiles[("xv", b)]
                ps = f_ps.tile([128, 512], f32, tag="f", name="fps")
                for e in range(NE):
                    nc.tensor.matmul(ps[:, 0:256],
                                     x_t[:, e, loc * 128:(loc + 1) * 128],
                                     wv_sb[:, e, :],
                                     start=(e == 0), stop=(e == NE - 1))
                est["pe"] += 8 * 118
                nc.vector.tensor_tensor(
                    out=v_sb[:, st, :, 0:DK],
                    in0=ps[:, 0:256].rearrange("p (h d) -> p h d", h=HPC),
                    in1=bv_bc[:, :, :],
                    op=mybir.AluOpType.add)

            def emit_y(J, o, half):
                jj = slice(J * 1024 + half * 512, J * 1024 + half * 512 + 512)
                ps = f_ps.tile([128, 512], f32, tag="f", name="fps")
                for f in range(2):
                    nc.tensor.matmul(ps[:, :], wo_sb[:, f, o * 128:(o + 1) * 128],
                                     xa_sb[:, f, jj],
                                     start=(f == 0), stop=(f == 1))
                est["pe"] += 2 * 235
                y_sb = y_pool.tile([128, 512], bf16, tag="y", name="ysb")
                nc.vector.tensor_scalar_add(y_sb[:, :], ps[:, :],
                                            bo_sb[:, o:o + 1])
                nc.sync.dma_start(y_t[o * 128:(o + 1) * 128, jj], y_sb[:, :])

            def emit_scores_exp(u, U, i):
                J, h = U
                sc = sc_ps.tile([128, 1024], f32, tag="sc", name="scps")
                ks = slice(i * 128, (i + 1) * 128)
                for half in range(2):
                    jj = slice(J * 1024 + half * 512, J * 1024 + half * 512 + 512)
                    nc.tensor.matmul(sc[:, half * 512:half * 512 + 512],
                                     kt_sb[:, h, ks], qt_sb[:, h, jj],
                                     start=True, stop=True)
                est["pe"] += 470
                slot = u % NPST
                pst_slot[(U, i)] = slot
                nc.scalar.activation(pst[:, slot, :], sc[:, :],
                                     mybir.ActivationFunctionType.Exp,
                                     scale=0.125)
                est["sc"] = max(est["sc"], est["pe"] + 400) + 1077

            def emit_norm(U):
                # softmax denominators ride along as o_ps row 64; copy that
                # row out, broadcast it across 64 partitions on gpsimd, and
                # normalize with a single DVE divide (PSUM in0, SBUF in1).
                J, h = U
                Js = slice(J * 1024, J * 1024 + 1024)
                dn = dn_pool.tile([1, 1024], f32, tag="dn", name="dn")
                nc.vector.tensor_copy(dn[0:1, :], o_tile[0][DK:DK + 1, :])
                nc.vector.reciprocal(dn[0:1, :], dn[0:1, :])
                db = db_pool.tile([64, 1024], f32, tag="db", name="db")
                nc.gpsimd.partition_broadcast(db[:, :], dn[0:1, :])
                pb = 64 * (h % 2)
                nc.vector.tensor_tensor(
                    out=xa_sb[pb:pb + DK, h // 2, Js], in0=o_tile[0][0:DK, :],
                    in1=db[:, :], op=mybir.AluOpType.mult)

            def emit_pv(U, i):
                J, h = U
                emit_v(i)
                if i == 0:
                    o_tile[0] = o_ps.tile([DK + 1, 1024], f32, tag="o",
                                          name="ops")
                slot = pst_slot[(U, i)]
                for half in range(2):
                    hs = slice(half * 512, half * 512 + 512)
                    nc.tensor.matmul(o_tile[0][:, hs], v_sb[:, i, h, :],
                                     pst[:, slot, hs],
                                     start=(i == 0), stop=(i == NST - 1))
                est["pe"] += 470
                if i == NST - 1:
                    emit_norm(U)

            # ---- filler queue: (ready_ns, fn) in strict FIFO order ----
            # ready = conservative DMA-landing estimate (cumulative bytes at
            # ~0.35 MiB/us behind a ~9us fixed runtime startup).
            from collections import deque
            filler = deque()
            filler.append((17000, lambda: emit_chain("q", 0, 1)))
            filler.append((24000, lambda: emit_chain("k", 0, 1)))
            filler.append((29500, lambda: emit_chain("k", 1, 0)))
            filler.append((29500, lambda: emit_chain("k", 1, 1)))
            for st in range(NST):
                ready = {0: 34000, 1: 37000, 2: 40000, 3: 43000}[st // 4]
                filler.append((ready, lambda st=st: emit_v(st)))
            filler.append((50000, lambda: emit_chain("q", 1, 0)))
            filler.append((50000, lambda: emit_chain("q", 1, 1)))

            def pop_filler_if_slack(aggressive=False):
                while filler:
                    ready, fn = filler[0]
                    if not aggressive and est["pe"] + 500 > est["sc"]:
                        break
                    if ready > est["pe"]:
                        break
                    filler.popleft()
                    fn()

            # ---- prologue: block-0 Q/K chains for heads 0,1 ----
            emit_chain("q", 0, 0)
            emit_chain("k", 0, 0)

            # ---- backbone ----
            units = [(J, h) for J in range(2) for h in range(HPC)]
            exp_seq = [(U, i) for U in units for i in range(NST)]

            for u, (U, i) in enumerate(exp_seq):
                J, h = U
                # gates: chains this scores tile depends on
                emit_chain("q", J, h // 2)
                emit_chain("k", i // 8, h // 2)
                if u >= LAG:
                    emit_pv(*exp_seq[u - LAG])
                    Uc = exp_seq[u - LAG][0]
                    if exp_seq[u - LAG][1] == NST - 1:
                        if Uc == (0, 3):
                            for o in range(8):
                                for hf in range(2):
                                    filler.append(
                                        (0, lambda o=o, hf=hf: emit_y(0, o, hf)))
                pop_filler_if_slack(aggressive=(u >= 104))
                emit_scores_exp(u, U, i)
                if u == 6 and not xq_b1_emitted[0]:
                    xq_b1_emitted[0] = True
                    emit_x_dmas("xq", xq_pool, xq_t, 1)

            # ---- epilogue: drain PV, filler, then final out-projection ----
            for u in range(len(exp_seq) - LAG, len(exp_seq)):
                emit_pv(*exp_seq[u])
            while filler:
                _, fn = filler.popleft()
                fn()
            for o in range(8):
                for hf in range(2):
                    emit_y(1, o, hf)

    nc.finalize()
    return nc


def _get_built():
    global _BUILT
    if _BUILT is None:
        _BUILT = _build()
    return _BUILT


def kernel(**inputs):
    global LAST_EXEC_NS, LAST_RESULTS
    import ml_dtypes
    from concourse import bass_utils

    bf16 = ml_dtypes.bfloat16
    inp = {k: np.ascontiguousarray(np.asarray(v), dtype=np.float32)
           for k, v in inputs.items()}

    # host: t-bias MLP, folded into the K-projection bias
    t = inp["t"].reshape(B)
    h1 = np.maximum(inp["tW1"][:, 0][None, :] * t[:, None] + inp["tb1"][None, :], 0.0)
    t_bias = h1 @ inp["tW2"].T + inp["tb2"][None, :]          # [B, DK]

    in_maps = []
    for c in range(NCORES):
        b, g = c // 4, c % 4
        sl = slice(g * DPC, (g + 1) * DPC)
        bo_full = inp["bo"] if g == 0 else np.zeros(D, np.float32)
        in_maps.append({
            "xq_t": np.ascontiguousarray(inp["query"][b].T.astype(bf16)),
            "xk_t": np.ascontiguousarray(inp["key"][b].T.astype(bf16)),
            "xv_t": np.ascontiguousarray(inp["value"][b].T.astype(bf16)),
            "wq_t": np.ascontiguousarray(inp["Wq"][sl, :].T.astype(bf16)),
            "wk_t": np.ascontiguousarray(inp["Wk"][sl, :].T.astype(bf16)),
            "wv_t": np.ascontiguousarray(inp["Wv"][sl, :].T.astype(bf16)),
            "wo_t": np.ascontiguousarray(inp["Wo"][:, sl].T.astype(bf16)),
            "bq2": inp["bq"][sl].reshape(HPC, DK).copy(),
            "bk2": (inp["bk"][sl] + np.tile(t_bias[b], HPC)).reshape(HPC, DK),
            "bv1": inp["bv"][sl].reshape(1, DPC).copy(),
            "bo8": bo_full.reshape(8, 128).copy(),
        })

    nc = _get_built()
    if TRACE:
        _install_ntff_shim()
    try:
        res = bass_utils.run_bass_kernel_spmd(
            nc, in_maps, core_ids=list(range(NCORES)), trace=TRACE)
    except Exception:
        # transient device-unrecoverable states have been observed on a
        # first run; one retry on a fresh execute context clears them
        import time
        time.sleep(2.0)
        res = bass_utils.run_bass_kernel_spmd(
            nc, in_maps, core_ids=list(range(NCORES)), trace=False)
    LAST_EXEC_NS = res.exec_time_ns
    LAST_RESULTS = res

    out = np.zeros((B, S, D), np.float32)
    for c in range(NCORES):
        out[c // 4] += res.results[c]["y_t"].astype(np.float32).T
    return out


# revision 12
# speedup vs baseline: 1.2340x; 1.2340x over previous
"""Trainium2 Bass kernel for nn_MultiHeadedAttention (B=2, S=2048, D=1024, H=16).

Sharding: batch (2) x head-groups (4) -> 8 cores. Core c handles batch c//4,
heads [4*(c%4), 4*(c%4)+4).

v2: software-pipelined emission. The Scalar engine's exp stream (128 tiles of
[128kpos, 1024q], ~138us total) is the backbone; PE interleaves scores
(producer, 2x512-row matmuls per tile), PV (consumer, lagged LAG tiles), and
all projection / V / output-projection work as filler so PE never idles.
Inputs stream in e-chunk granularity DMAs ordered by first use; output
partials are emitted in bf16 (host sums in fp32).

Per-core math identical to baseline:
  Q^T/K^T projections land heads on partitions 0:64 (rows 64:128 zeroed so
  scores matmuls contract K=128); V in [seq, feat] layout with a ones column
  per head so PV's 65th output row accumulates the softmax denominators.
  exp on ScalarE with the 1/sqrt(dk) scale folded in (no max-subtraction:
  |scores| <~ 8 is safe). Normalization: reciprocal on a [128,8] lane-parallel
  reshape via DRAM bounce + broadcast DMA + one vector multiply.
  t-bias MLP folded into the K bias on host.
"""

import numpy as np

B, S, D, H, DK = 2, 2048, 1024, 16, 64
HPC = 4            # heads per core
DPC = HPC * DK     # 256 features per core
NCORES = 8

TRACE = False          # test harness sets True to capture an NTFF profile
LAST_EXEC_NS = None    # filled when TRACE
LAST_RESULTS = None

_BUILT = None


def _install_ntff_shim():
    """antenv.axon_hooks is absent in this image; recreate it so trace=True
    can ship NTFF profiles back through the axon tunnel."""
    import sys, types
    try:
        from antenv import axon_hooks  # noqa: F401
        return
    except ImportError:
        pass
    import antenv
    mod = types.ModuleType("antenv.axon_hooks")
    _hook = [None]
    mod.set_axon_ntff_profile_hook = lambda h: _hook.__setitem__(0, h)
    mod.get_axon_ntff_profile_hook = lambda: _hook[0]
    sys.modules["antenv.axon_hooks"] = mod
    antenv.axon_hooks = mod
    try:
        from trn_agent_boot.trn_boot import _ntff_profile_via_ctypes
        mod.set_axon_ntff_profile_hook(
            _ntff_profile_via_ctypes("/opt/axon/libaxon_pjrt.so"))
    except Exception:
        pass


def _build():
    """Build the per-core Bass graph (identical on all 8 cores)."""
    import concourse.tile as tile
    from concourse import mybir, bacc

    f32 = mybir.dt.float32
    bf16 = mybir.dt.bfloat16

    nc = bacc.Bacc()

    xq_t = nc.dram_tensor("xq_t", [D, S], bf16, kind="ExternalInput")
    xk_t = nc.dram_tensor("xk_t", [D, S], bf16, kind="ExternalInput")
    xv_t = nc.dram_tensor("xv_t", [D, S], bf16, kind="ExternalInput")
    wq_t = nc.dram_tensor("wq_t", [D, DPC], bf16, kind="ExternalInput")
    wk_t = nc.dram_tensor("wk_t", [D, DPC], bf16, kind="ExternalInput")
    wv_t = nc.dram_tensor("wv_t", [D, DPC], bf16, kind="ExternalInput")
    wo_t = nc.dram_tensor("wo_t", [DPC, D], bf16, kind="ExternalInput")
    bq2 = nc.dram_tensor("bq2", [HPC, DK], f32, kind="ExternalInput")
    bk2 = nc.dram_tensor("bk2", [HPC, DK], f32, kind="ExternalInput")
    bv1 = nc.dram_tensor("bv1", [1, DPC], f32, kind="ExternalInput")
    bo8 = nc.dram_tensor("bo8", [8, 128], f32, kind="ExternalInput")
    y_t = nc.dram_tensor("y_t", [D, S], bf16, kind="ExternalOutput")

    NE = D // 128   # 8 feature chunks
    NST = S // 128  # 16 seq k-tiles of 128
    NPST = 16       # exp staging depth (p tiles in flight)
    LAG = 14        # PV trails the exp backbone by this many tiles

    with tile.TileContext(nc) as tc:
        with tc.tile_pool(name="consts", bufs=1) as consts, \
             tc.tile_pool(name="persist", bufs=1) as persist, \
             tc.tile_pool(name="xq_pool", bufs=1) as xq_pool, \
             tc.tile_pool(name="xk_pool", bufs=2) as xk_pool, \
             tc.tile_pool(name="xv_pool", bufs=2) as xv_pool, \
             tc.tile_pool(name="oasb", bufs=2) as oa_pool, \
             tc.tile_pool(name="dnsb", bufs=2) as dn_pool, \
             tc.tile_pool(name="dbsb", bufs=2) as db_pool, \
             tc.tile_pool(name="ysb", bufs=4) as y_pool, \
             tc.tile_pool(name="sc_ps", bufs=2, space="PSUM") as sc_ps, \
             tc.tile_pool(name="o_ps", bufs=1, space="PSUM") as o_ps, \
             tc.tile_pool(name="f_ps", bufs=2, space="PSUM") as f_ps:

            # ---- persistent activations ----
            qt_sb = persist.tile([128, HPC, S], bf16, tag="qt")
            kt_sb = persist.tile([128, HPC, S], bf16, tag="kt")
            # rows 64:128 zeroed so scores matmuls contract K=128
            # (FWL-eligible weights); split across engines so both finish
            # ~7us after queue start and neither blocks chain evacuations.
            nc.gpsimd.memset(qt_sb[64:128, :, :], 0.0)
            nc.vector.memset(kt_sb[64:128, :, :], 0.0)
            v_sb = persist.tile([128, NST, HPC, DK + 1], bf16, tag="v")
            pst = persist.tile([128, NPST, 1024], bf16, tag="pst")
            xa_sb = persist.tile([128, 2, S], bf16, tag="xa")
            ones1 = consts.tile([128, 1], f32, tag="ones1")
            nc.vector.memset(ones1[:, :], 1.0)
            nc.vector.tensor_copy(
                v_sb[:, :, :, DK:DK + 1].rearrange("p a b c -> p (a b c)"),
                ones1[:, 0:1].broadcast_to([128, NST * HPC]))

            # ---- weights + inputs: DMA emission order is priority order ----
            wq_sb = consts.tile([128, NE, DPC], bf16, tag="wq")
            nc.sync.dma_start(wq_sb[:, :, :],
                              wq_t.rearrange("(e p) n -> p e n", p=128))
            x_tiles = {}

            def emit_x_dmas(name, pool, dram, b, halves=1):
                t = pool.tile([128, NE, 1024], bf16, tag=name, name=name)
                x_tiles[(name, b)] = t
                src = dram.rearrange("(e p) s -> p e s", p=128)
                cw = 1024 // halves
                for hf in range(halves):
                    for e in range(NE):
                        nc.sync.dma_start(
                            t[:, e, hf * cw:(hf + 1) * cw],
                            src[:, e, b * 1024 + hf * cw:b * 1024 + (hf + 1) * cw])

            def emit_xv_half(b, hf):
                # column-half loads so V-projection st tiles unblock early
                name = ("xv", b)
                if name not in x_tiles:
                    x_tiles[name] = xv_pool.tile([128, NE, 1024], bf16,
                                                 tag="xv", name="xv")
                t = x_tiles[name]
                src = xv_t.rearrange("(e p) s -> p e s", p=128)
                for e in range(NE):
                    nc.sync.dma_start(
                        t[:, e, hf * 512:(hf + 1) * 512],
                        src[:, e, b * 1024 + hf * 512:b * 1024 + (hf + 1) * 512])

            emit_x_dmas("xq", xq_pool, xq_t, 0)
            wk_sb = consts.tile([128, NE, DPC], bf16, tag="wk")
            nc.sync.dma_start(wk_sb[:, :, :],
                              wk_t.rearrange("(e p) n -> p e n", p=128))
            bq_sb = consts.tile([64, HPC], f32, tag="bq")
            bk_sb = consts.tile([64, HPC], f32, tag="bk")
            nc.sync.dma_start(bq_sb[:, :], bq2.rearrange("h p -> p h"))
            nc.sync.dma_start(bk_sb[:, :], bk2.rearrange("h p -> p h"))
            emit_x_dmas("xk", xk_pool, xk_t, 0)
            emit_x_dmas("xk", xk_pool, xk_t, 1)
            wv_sb = consts.tile([128, NE, DPC], bf16, tag="wv")
            nc.sync.dma_start(wv_sb[:, :, :],
                              wv_t.rearrange("(e p) n -> p e n", p=128))
            bv_bc = consts.tile([128, HPC, DK], f32, tag="bvb")
            nc.sync.dma_start(
                bv_bc.rearrange("p h d -> p (h d)"),
                bv1[0:1, :].broadcast_to([128, DPC]))
            emit_xv_half(0, 0)
            emit_xv_half(0, 1)
            emit_xv_half(1, 0)
            emit_xv_half(1, 1)
            wo_sb = consts.tile([128, 2, D], bf16, tag="wo")
            nc.sync.dma_start(wo_sb[:, :, :],
                              wo_t.rearrange("(f p) n -> p f n", p=128))
            bo_sb = consts.tile([128, 8], f32, tag="bo")
            nc.sync.dma_start(bo_sb[:, :], bo8.rearrange("o p -> p o"))
            # xq block 1 DMAs are deferred (xq_pool bufs=1, SBUF pressure):
            # emitted mid-driver once block-0 Q chains are in the queue.

            # ---- emission state ----
            est = {"pe": 7000.0, "sc": 0.0}
            xq_b1_emitted = [False]
            chain_done = set()   # ('q'|'k', block, m)
            v_done = set()
            pst_slot = {}
            o_tile = [None]

            def emit_chain(kind, b, m):
                if (kind, b, m) in chain_done:
                    return
                chain_done.add((kind, b, m))
                if kind == "q" and b == 1 and ("xq", 1) not in x_tiles:
                    xq_b1_emitted[0] = True
                    emit_x_dmas("xq", xq_pool, xq_t, 1)
                x_t = x_tiles[("xq" if kind == "q" else "xk", b)]
                w_sb = wq_sb if kind == "q" else wk_sb
                dst = qt_sb if kind == "q" else kt_sb
                bias = bq_sb if kind == "q" else bk_sb
                ms = slice(m * 128, m * 128 + 128)
                ps0 = f_ps.tile([128, 512], f32, tag="f", name="fps")
                ps1 = f_ps.tile([128, 512], f32, tag="f", name="fps")
                for e in range(NE):
                    nc.tensor.matmul(ps0[:, :], w_sb[:, e, ms],
                                     x_t[:, e, 0:512],
                                     start=(e == 0), stop=(e == NE - 1))
                    nc.tensor.matmul(ps1[:, :], w_sb[:, e, ms],
                                     x_t[:, e, 512:1024],
                                     start=(e == 0), stop=(e == NE - 1))
                est["pe"] += 16 * 235
                for half, ps in enumerate((ps0, ps1)):
                    sl = slice(b * 1024 + half * 512, b * 1024 + half * 512 + 512)
                    nc.vector.tensor_scalar_add(
                        dst[0:64, 2 * m, sl], ps[0:64, :],
                        bias[:, 2 * m:2 * m + 1])
                    nc.vector.tensor_scalar_add(
                        dst[0:64, 2 * m + 1, sl], ps[64:128, :],
                        bias[:, 2 * m + 1:2 * m + 2])

            def emit_v(st):
                if st in v_done:
                    return
                v_done.add(st)
                b, loc = st // 8, st % 8
                x_t = x_tiles[("xv", b)]
                ps = f_ps.tile([128, 512], f32, tag="f", name="fps")
                for e in range(NE):
                    nc.tensor.matmul(ps[:, 0:256],
                                     x_t[:, e, loc * 128:(loc + 1) * 128],
                                     wv_sb[:, e, :],
                                     start=(e == 0), stop=(e == NE - 1))
                est["pe"] += 8 * 118
                nc.vector.tensor_tensor(
                    out=v_sb[:, st, :, 0:DK],
                    in0=ps[:, 0:256].rearrange("p (h d) -> p h d", h=HPC),
                    in1=bv_bc[:, :, :],
                    op=mybir.AluOpType.add)

            def emit_y(J, o, half):
                jj = slice(J * 1024 + half * 512, J * 1024 + half * 512 + 512)
                ps = f_ps.tile([128, 512], f32, tag="f", name="fps")
                for f in range(2):
                    nc.tensor.matmul(ps[:, :], wo_sb[:, f, o * 128:(o + 1) * 128],
                                     xa_sb[:, f, jj],
                                     start=(f == 0), stop=(f == 1))
                est["pe"] += 2 * 235
                y_sb = y_pool.tile([128, 512], bf16, tag="y", name="ysb")
                nc.vector.tensor_scalar_add(y_sb[:, :], ps[:, :],
                                            bo_sb[:, o:o + 1])
                nc.sync.dma_start(y_t[o * 128:(o + 1) * 128, jj], y_sb[:, :])

            def emit_scores_exp(u, U, i):
                J, h = U
                sc = sc_ps.tile([128, 1024], f32, tag="sc", name="scps")
                ks = slice(i * 128, (i + 1) * 128)
                for half in range(2):
                    jj = slice(J * 1024 + half * 512, J * 1024 + half * 512 + 512)
                    nc.tensor.matmul(sc[:, half * 512:half * 512 + 512],
                                     kt_sb[:, h, ks], qt_sb[:, h, jj],
                                     start=True, stop=True)
                est["pe"] += 470
                slot = u % NPST
                pst_slot[(U, i)] = slot
                nc.scalar.activation(pst[:, slot, :], sc[:, :],
                                     mybir.ActivationFunctionType.Exp,
                                     scale=0.125)
                est["sc"] = max(est["sc"], est["pe"] + 400) + 1077

            def emit_norm(U):
                # softmax denominators ride along as o_ps row 64; copy that
                # row out, broadcast it across 64 partitions on gpsimd, and
                # normalize with a single DVE divide (PSUM in0, SBUF in1).
                J, h = U
                Js = slice(J * 1024, J * 1024 + 1024)
                # evacuate PSUM immediately so the next unit's PV can
                # claim the o_ps slot without waiting on the norm chain
                oa = oa_pool.tile([DK + 1, 1024], f32, tag="oa", name="oa")
                nc.vector.tensor_copy(oa[:, :], o_tile[0][:, :])
                dn = dn_pool.tile([1, 1024], f32, tag="dn", name="dn")
                nc.vector.tensor_copy(dn[0:1, :], oa[DK:DK + 1, :])
                nc.vector.reciprocal(dn[0:1, :], dn[0:1, :])
                db = db_pool.tile([64, 1024], f32, tag="db", name="db")
                nc.gpsimd.partition_broadcast(db[:, :], dn[0:1, :])
                pb = 64 * (h % 2)
                nc.vector.tensor_tensor(
                    out=xa_sb[pb:pb + DK, h // 2, Js], in0=oa[0:DK, :],
                    in1=db[:, :], op=mybir.AluOpType.mult)

            def emit_pv(U, i):
                J, h = U
                emit_v(i)
                if i == 0:
                    o_tile[0] = o_ps.tile([DK + 1, 1024], f32, tag="o",
                                          name="ops")
                slot = pst_slot[(U, i)]
                for half in range(2):
                    hs = slice(half * 512, half * 512 + 512)
                    nc.tensor.matmul(o_tile[0][:, hs], v_sb[:, i, h, :],
                                     pst[:, slot, hs],
                                     start=(i == 0), stop=(i == NST - 1))
                est["pe"] += 470
                if i == NST - 1:
                    emit_norm(U)

            # ---- filler queue: (ready_ns, fn) in strict FIFO order ----
            # ready = conservative DMA-landing estimate (cumulative bytes at
            # ~0.35 MiB/us behind a ~9us fixed runtime startup).
            from collections import deque
            filler = deque()
            filler.append((17000, lambda: emit_chain("q", 0, 1)))
            filler.append((24000, lambda: emit_chain("k", 0, 1)))
            filler.append((29500, lambda: emit_chain("k", 1, 0)))
            filler.append((29500, lambda: emit_chain("k", 1, 1)))
            for st in range(NST):
                ready = {0: 34000, 1: 37000, 2: 40000, 3: 43000}[st // 4]
                filler.append((ready, lambda st=st: emit_v(st)))
            filler.append((50000, lambda: emit_chain("q", 1, 0)))
            filler.append((50000, lambda: emit_chain("q", 1, 1)))

            def pop_filler_if_slack(aggressive=False):
                while filler:
                    ready, fn = filler[0]
                    if not aggressive and est["pe"] + 500 > est["sc"]:
                        break
                    if ready > est["pe"]:
                        break
                    filler.popleft()
                    fn()

            # ---- prologue: block-0 Q/K chains for heads 0,1 ----
            emit_chain("q", 0, 0)
            emit_chain("k", 0, 0)

            # ---- backbone ----
            units = [(J, h) for J in range(2) for h in range(HPC)]
            exp_seq = [(U, i) for U in units for i in range(NST)]

            for u, (U, i) in enumerate(exp_seq):
                J, h = U
                # gates: chains this scores tile depends on
                emit_chain("q", J, h // 2)
                emit_chain("k", i // 8, h // 2)
                if u >= LAG:
                    emit_pv(*exp_seq[u - LAG])
                    Uc = exp_seq[u - LAG][0]
                    if exp_seq[u - LAG][1] == NST - 1:
                        if Uc == (0, 3):
                            for o in range(8):
                                for hf in range(2):
                                    filler.append(
                                        (0, lambda o=o, hf=hf: emit_y(0, o, hf)))
                pop_filler_if_slack(aggressive=(u >= 104))
                emit_scores_exp(u, U, i)
                if u == 6 and not xq_b1_emitted[0]:
                    xq_b1_emitted[0] = True
                    emit_x_dmas("xq", xq_pool, xq_t, 1)

            # ---- epilogue: drain PV, filler, then final out-projection ----
            for u in range(len(exp_seq) - LAG, len(exp_seq)):
                emit_pv(*exp_seq[u])
            while filler:
                _, fn = filler.popleft()
                fn()
            for o in range(8):
                for hf in range(2):
                    emit_y(1, o, hf)

    nc.finalize()
    return nc


def _get_built():
    global _BUILT
    if _BUILT is None:
        _BUILT = _build()
    return _BUILT


def kernel(**inputs):
    global LAST_EXEC_NS, LAST_RESULTS
    import ml_dtypes
    from concourse import bass_utils

    bf16 = ml_dtypes.bfloat16
    inp = {k: np.ascontiguousarray(np.asarray(v), dtype=np.float32)
           for k, v in inputs.items()}

    # host: t-bias MLP, folded into the K-projection bias
    t = inp["t"].reshape(B)
    h1 = np.maximum(inp["tW1"][:, 0][None, :] * t[:, None] + inp["tb1"][None, :], 0.0)
    t_bias = h1 @ inp["tW2"].T + inp["tb2"][None, :]          # [B, DK]

    in_maps = []
    for c in range(NCORES):
        b, g = c // 4, c % 4
        sl = slice(g * DPC, (g + 1) * DPC)
        bo_full = inp["bo"] if g == 0 else np.zeros(D, np.float32)
        in_maps.append({
            "xq_t": np.ascontiguousarray(inp["query"][b].T.astype(bf16)),
            "xk_t": np.ascontiguousarray(inp["key"][b].T.astype(bf16)),
            "xv_t": np.ascontiguousarray(inp["value"][b].T.astype(bf16)),
            "wq_t": np.ascontiguousarray(inp["Wq"][sl, :].T.astype(bf16)),
            "wk_t": np.ascontiguousarray(inp["Wk"][sl, :].T.astype(bf16)),
            "wv_t": np.ascontiguousarray(inp["Wv"][sl, :].T.astype(bf16)),
            "wo_t": np.ascontiguousarray(inp["Wo"][:, sl].T.astype(bf16)),
            "bq2": inp["bq"][sl].reshape(HPC, DK).copy(),
            "bk2": (inp["bk"][sl] + np.tile(t_bias[b], HPC)).reshape(HPC, DK),
            "bv1": inp["bv"][sl].reshape(1, DPC).copy(),
            "bo8": bo_full.reshape(8, 128).copy(),
        })

    nc = _get_built()
    if TRACE:
        _install_ntff_shim()
    try:
        res = bass_utils.run_bass_kernel_spmd(
            nc, in_maps, core_ids=list(range(NCORES)), trace=TRACE)
    except Exception:
        # transient device-unrecoverable states have been observed on a
        # first run; one retry on a fresh execute context clears them
        import time
        time.sleep(2.0)
        res = bass_utils.run_bass_kernel_spmd(
            nc, in_maps, core_ids=list(range(NCORES)), trace=False)
    LAST_EXEC_NS = res.exec_time_ns
    LAST_RESULTS = res

    out = np.zeros((B, S, D), np.float32)
    for c in range(NCORES):
        out[c // 4] += res.results[c]["y_t"].astype(np.float32).T
    return out
